# revision 96
# baseline (speedup 1.0000x reference)
"""Trainium2 Bass kernel for nn_Attention_9423158248136 — v2.

Attention: B=4, N=2048, D=512, H=8, DH=64, full [H, N, N] additive bias
inside softmax.  Head-parallel: core c owns head c for all 4 batches; the
8 partial row-split outputs are summed on the host gather path.

v2 redesign vs the 200us baseline:
  - bias is added INTO the QK PSUM by an fp8e4m3 DoubleRow identity matmul
    streaming host-precomputed 8*log1p(bias); exp(0.125 * psum) on ACT then
    yields the exact softmax numerator — no DVE bias multiply, no 1+bias
    approximation, and ACT does nothing but the 64 exp instructions.
  - q/k are quantized to fp8e4m3 and both the qk projection and QK^T run in
    DoubleRow perf mode (0.5 cycles/row), halving their PE time.
  - exp operates on [128, 2048] quad tiles (4 PSUM banks, single buffer):
    64 activations, ACT busy ~125us — the kernel's floor.
  - AV uses p as the STATIONARY operand ([128 j, 128 i] tiles) with moving
    V augmented by a ones column: full 128-wide PE output (30us vs 55us),
    and the row-sums land per-partition in the aug column — softmax
    normalization is one reciprocal + per-partition scale, killing the
    DRAM bounce and the post-projection scaling pass of the baseline.
  - attn output [i, d] tiles are PE-transposed back to [d, i] for the
    output projection; projections and epilogue are spread across DVE
    while ACT stays exp-only.
  - the per-chain AV accumulator is ONE psum bank holding all four
    interleaved i-blocks (DVE-memset + start=False accumulation), so AV
    partials run in the same ih-chunk as their exps and the pipeline tail
    collapses to one quad's worth of work.
  - DMA is scheduled by first-use time on the serial DGE resource: x/bias/
    shuffle transfers interleaved so the ACT stream starts ~11us in and
    runs gapless; bias is stored two seg-pairs per partition row so its
    transfers run at full 128-partition rate.

v2.1: pout partials stored/gathered in fp16 (0.05%% quantization vs
bf16's 0.4%% on the 8-way host sum) -- same schedule, same 154.0us.

v2.2 (150.1us): multi-sequencer DMA issue.  The 11us startup was bound
by ~9 serialized dma_start issues (~1.2us each: SP seq + HWDGE + DGE
delay).  DMAs can also be initiated from the Activation and GpSimd
sequencers: id8/wv/idb/wout and one rb0 shuffle issue from ACT (idle
until its first exp), and ALL b8 bias loads + q/k shuffles + odd x
loads issue from the always-idle GpSimd sequencer.  This parallelizes
the issue path 3-way at startup and decongests the SP queue against the
pout writebacks for the rest of the kernel: first exp ~8us (was ~11),
wall 154.0 -> 150.1us.

Verification: relative error 1.30e-2 vs the fp32 reference on silicon
(8 cores via run_bass_kernel_spmd / PJRT); cost-model schedule 154.0us
vs 200.1us for the previous baseline.  ACT (exp) busy 134.2us with zero
mid-stream gaps — the engine floor for this decomposition; PE ~122us,
DVE ~88us, DMA ~93us.  attnT is kept unnormalized (bf16-safe, values
O(25)); the 1/rowsum scale rides every out-projection evacuation as a
per-partition TensorScalarPtr at identical cost.
"""

import numpy as np
import ml_dtypes

import concourse.bass as bass
import concourse.mybir as mybir
from concourse import tile
from concourse.bass_utils import run_bass_kernel_spmd

B, N, D = 4, 2048, 512
H, DH = 8, 64
ROWS = B * N  # 8192
NCORES = 8
NJ = N // 128        # 16 key blocks per batch
IH = 512             # queries per ih-chunk
NGI = B * (N // IH)  # 16 global ih chunks
NQUAD = NGI * 4      # 64 quad tiles

FP32 = mybir.dt.float32
BF16 = mybir.dt.bfloat16
FP16 = mybir.dt.float16
FP8 = mybir.dt.float8e4
BF16_NP = ml_dtypes.bfloat16
F8_NP = ml_dtypes.float8_e4m3
DR = mybir.MatmulPerfMode.DoubleRow

LAST_RESULTS = None


def build_nc():
    nc = bass.Bass()

    xT = nc.declare_dram_parameter("xT", [D, ROWS], BF16, isOutput=False)
    # wqk col = kb*128 + m  (rows kb*128+p of the [512, 128] q|k weight)
    wqk = nc.declare_dram_parameter("wqk", [128, 4 * 128], BF16, isOutput=False)
    wv = nc.declare_dram_parameter("wv", [D, DH], BF16, isOutput=False)
    wout = nc.declare_dram_parameter("wout", [DH, D], BF16, isOutput=False)
    # identity replicated on both partition halves (seg pairs use either)
    id8 = nc.declare_dram_parameter("id8", [128, 256], FP8, isOutput=False)
    idb = nc.declare_dram_parameter("idb", [128, 128], BF16, isOutput=False)
    # b8 [128, 32768]: partition 64*(s//2)+p, col = u*2048 + (s%2)*1024
    #   + t2*512 + il ; j = (4t+s)*128 + p + 64*t2  (128-partition DMA rate)
    b8 = nc.declare_dram_parameter("b8", [128, 16 * 2048], FP8, isOutput=False)
    pout = nc.declare_dram_parameter("pout", [ROWS, D], FP16, isOutput=True)

    with tile.TileContext(nc) as tc:
        with (
            tc.tile_pool(name="qk_psumA", bufs=1, space="PSUM") as qk_poolA,
            tc.tile_pool(name="qk_psumB", bufs=1, space="PSUM") as qk_poolB,
            tc.tile_pool(name="aug_psum", bufs=2, space="PSUM") as aug_pool,
            tc.tile_pool(name="b1_psum", bufs=2, space="PSUM") as b1_pool,
            tc.tile_pool(name="consts", bufs=1) as consts,
            tc.tile_pool(name="xtiles", bufs=6) as xtiles,
            tc.tile_pool(name="atp", bufs=6) as atp,
            tc.tile_pool(name="rcp", bufs=3) as rcp,
            tc.tile_pool(name="pos", bufs=5) as pos,
        ):
            # ---- resident SBUF ----
            wqk_sb = consts.tile([128, 4 * 128], BF16, name="wqk_sb")
            wv_sb = consts.tile([128, 4 * DH], BF16, name="wv_sb")
            wout_sb = consts.tile([DH, D], BF16, name="wout_sb")
            id8_sb = consts.tile([128, 256], FP8, name="id8_sb")
            idb_sb = consts.tile([128, 128], BF16, name="idb_sb")
            qk8_sb = consts.tile([128, ROWS], FP8, name="qk8_sb")  # q 0:64 | k 64:128
            q8s = consts.tile([32, 2 * ROWS], FP8, name="q8s")     # col = t2*ROWS + r
            k8s = consts.tile([32, 2 * ROWS], FP8, name="k8s")     # col = t2*ROWS + r
            v_sb = consts.tile([128, B * NJ * (DH + 1)], BF16, name="v_sb")
            p_sb = consts.tile([128, 2 * 8192], BF16, name="p_sb")  # 2 gi slots
            attnT = consts.tile([DH, 2 * IH], BF16, name="attnT")   # 2 gi slots
            b8_sb = consts.tile([128, 4 * 8192], FP8, name="b8_sb")  # resident

            # ones columns of augmented V (strided memset at d=64 of each block)
            nc.vector.memset(
                v_sb[:].rearrange("p (blk d) -> p blk d", d=DH + 1)[:, :, DH:DH + 1],
                1.0)

            def b8_load(uq):
                # bias is batch-independent: 16 resident quad-slices, loaded
                # once each, ordered incrementally ahead of first use
                nc.gpsimd.dma_start(
                    b8_sb[:, uq * 2048:(uq + 1) * 2048],
                    b8[:, uq * 2048:(uq + 1) * 2048])

            # ---- per-row-block (512 rows) projections ----
            def xload(rb, split=False):
                xt = xtiles.tile([128, 4 * 512], BF16, name=f"xt{rb}", tag="xt")
                xTr = xT[:, rb * 512:(rb + 1) * 512].rearrange(
                    "(kb p) n -> p kb n", p=128)
                xtr = xt[:].rearrange("p (kb n) -> p kb n", kb=4)
                if split:
                    # two half-loads: the projection's first accumulation
                    # matmuls start as soon as the kb 0-1 half lands
                    nc.sync.dma_start(xtr[:, 0:2, :], xTr[:, 0:2, :])
                    nc.sync.dma_start(xtr[:, 2:4, :], xTr[:, 2:4, :])
                elif rb % 2 == 1:
                    nc.gpsimd.dma_start(xtr, xTr)
                else:
                    nc.sync.dma_start(xtr, xTr)
                return xt

            qkp_tiles = {}

            def qkproj(rb, xt, half=None):
                # half=0/1 splits the 4 accumulation matmuls across two
                # emission slots (drip pacing); half=None does all at once
                kbs = range(4) if half is None else range(2 * half, 2 * half + 2)
                if half in (None, 0):
                    qkp_tiles[rb] = b1_pool.tile([128, 512], FP32,
                                                 name=f"qkp{rb}", tag="b1")
                qkp = qkp_tiles[rb]
                for kb in kbs:
                    nc.tensor.matmul(
                        qkp[:],
                        wqk_sb[:, kb * 128:(kb + 1) * 128],
                        xt[:, kb * 512:(kb + 1) * 512],
                        start=(kb == 0), stop=(kb == 3))
                if half in (None, 1):
                    nc.vector.tensor_scalar_mul(
                        qk8_sb[:, rb * 512:(rb + 1) * 512], qkp[:], 1.0)
                    del qkp_tiles[rb]

            def vproj(rb, xt):
                vp = b1_pool.tile([128, 256], FP32, name=f"vp{rb}", tag="b1")
                for sub in range(4):
                    for kb in range(4):
                        nc.tensor.matmul(
                            vp[:, sub * DH:(sub + 1) * DH],
                            xt[:, kb * 512 + sub * 128:kb * 512 + (sub + 1) * 128],
                            wv_sb[:, kb * DH:(kb + 1) * DH],
                            start=(kb == 0), stop=(kb == 3))
                bi0 = rb * 4
                nc.vector.tensor_scalar_mul(
                    v_sb[:, bi0 * (DH + 1):(bi0 + 4) * (DH + 1)].rearrange(
                        "p (blk d) -> p blk d", d=DH + 1)[:, :, 0:DH],
                    vp[:].rearrange("p (blk d) -> p blk d", d=DH), 1.0)

            def shuffle(bb, rb=None, parts="qk", cols=None):
                # q/k fp8 partition interleave: d or j = p + 32*t2 halves.
                # rb!=None shuffles a single 512-row slice (head pipelining).
                if cols is not None:
                    c0, c1 = cols
                else:
                    c0, c1 = (bb * N, (bb + 1) * N) if rb is None else \
                        (rb * 512, (rb + 1) * 512)
                for t2 in range(2):
                    if "q" in parts:
                        nc.gpsimd.dma_start(
                            q8s[:, t2 * ROWS + c0:t2 * ROWS + c1],
                            qk8_sb[32 * t2:32 * (t2 + 1), c0:c1])
                    if "k" in parts:
                        nc.gpsimd.dma_start(
                            k8s[:, t2 * ROWS + c0:t2 * ROWS + c1],
                            qk8_sb[64 + 32 * t2:96 + 32 * t2, c0:c1])

            # ---- attention building blocks ----
            def qk_quad(u):
                """two independent [128, 1024] half-tiles per quad (pool
                bufs=2, 2 banks each): QK halves double-buffer against ACT —
                segs of half h of quad u+1 wait only on exp of half h of
                quad u."""
                b, gi_l, t = u // 16, (u // 4) % 4, u % 4
                gi = u // 4
                i0 = b * N + gi_l * IH
                k8r = k8s[:].rearrange("p (two r) -> p two r", two=2)
                q8r = q8s[:].rearrange("p (two r) -> p two r", two=2)
                bslot = (gi % 4) * 8192 + t * 2048
                pbase = (gi % 2) * 8192 + t * 2048

                def seg(half, s):
                    jb = 4 * t + s
                    h8 = s // 2
                    nc.tensor.matmul(
                        half[:, (s % 2) * 512:(s % 2 + 1) * 512],
                        id8_sb[64 * h8:64 * (h8 + 1), :].rearrange(
                            "p (two m) -> p two m", two=2),
                        b8_sb[64 * h8:64 * (h8 + 1),
                              bslot + (s % 2) * 1024:
                              bslot + (s % 2 + 1) * 1024].rearrange(
                            "p (two n) -> p two n", two=2),
                        start=True, stop=False, perf_mode=DR)
                    nc.tensor.matmul(
                        half[:, (s % 2) * 512:(s % 2 + 1) * 512],
                        k8r[:, :, b * N + jb * 128:b * N + (jb + 1) * 128],
                        q8r[:, :, i0:i0 + IH],
                        start=False, stop=True, perf_mode=DR)

                for h in range(2):
                    pool = qk_poolA if (2 * u + h) % 2 == 0 else qk_poolB
                    half = pool.tile([128, 1024], FP32,
                                     name=f"qk{u}_{h}", tag="qk")
                    seg(half, 2 * h)
                    seg(half, 2 * h + 1)
                    nc.scalar.activation(
                        p_sb[:, pbase + h * 1024:pbase + (h + 1) * 1024],
                        half[:], mybir.ActivationFunctionType.Exp, scale=0.125)

            at_tiles = {}   # (gi, m) -> at tile
            rc_tiles = {}   # (gi, m) -> recip tile (tail chains)
            aug_tiles = {}  # gi -> [128, 4*66] interleaved accumulator

            def av_memset(gi):
                # one aug bank per chain, 4 blocks at 66-stride; zeroed by
                # DVE so the interleaved chains can accumulate start=False
                aug = aug_pool.tile([128, 4 * 66], FP32, name=f"aug{gi}",
                                    tag="aug")
                nc.vector.memset(aug[:], 0.0)
                aug_tiles[gi] = aug

            def av_partial(gi, t):
                # AV contributions of quads t (j-blocks 4t..4t+3) for all 4
                # i-blocks of chain gi
                b = gi // 4
                base = (gi % 2) * 8192
                aug = aug_tiles[gi]
                for s in range(4):
                    jb = 4 * t + s
                    vcol = (b * NJ + jb) * (DH + 1)
                    for m in range(4):
                        pcol = base + t * 2048 + s * 512 + m * 128
                        nc.tensor.matmul(
                            aug[:, m * 66:m * 66 + DH + 1],
                            p_sb[:, pcol:pcol + 128],
                            v_sb[:, vcol:vcol + DH + 1],
                            start=False, stop=(jb == NJ - 1))

            def av_post_scales(gi):
                # attnT stays UNNORMALIZED: the 1/rowsum rides the out-proj
                # evacuation (per-partition there too), so the evac here is a
                # single strided copy of all four blocks
                aug = aug_tiles.pop(gi)
                augr = aug[:].rearrange("p (m c) -> p m c", c=66)
                rc4 = rcp.tile([128, 4], FP32, name=f"rc{gi}", tag="rc")
                nc.vector.reciprocal(rc4[:], augr[:, :, DH:DH + 1])
                at4 = atp.tile([128, 4 * DH], BF16, name=f"at{gi}", tag="at")
                nc.vector.tensor_scalar_mul(
                    at4[:].rearrange("p (m d) -> p m d", d=DH),
                    augr[:, :, 0:DH], 1.0)
                for m in range(4):
                    rc_tiles[(gi, m)] = rc4
                    at_tiles[(gi, m)] = at4
            def transpose_ih(gi):
                tp = b1_pool.tile([DH, 512], BF16, name=f"tp{gi}", tag="b1")
                for m in range(4):
                    at4 = at_tiles.pop((gi, m))
                    nc.tensor.transpose(
                        tp[:, m * 128:(m + 1) * 128],
                        at4[:, m * DH:(m + 1) * DH], idb_sb[:])
                nc.vector.tensor_copy(
                    attnT[:, (gi % 2) * 512:(gi % 2 + 1) * 512], tp[:])

            pst_tiles = {}
            pout_queue = []

            def pout_flush():
                g = pout_queue.pop(0)
                nc.sync.dma_start(
                    pout[:].rearrange("(r p) d -> p r d", p=128)
                        [:, g * 4:g * 4 + 4, :],
                    pst_tiles[g][:].rearrange("p (m d) -> p m d", m=4))
                del pst_tiles[g]

            def outproj_block(gi, m, last=None):
                op = b1_pool.tile([128, 512], FP32, name=f"op{gi}_{m}", tag="b1")
                nc.tensor.matmul(
                    op[:],
                    attnT[:, (gi % 2) * 512 + m * 128:(gi % 2) * 512 + (m + 1) * 128],
                    wout_sb[:], start=True, stop=True)
                if last is not None:
                    # tail: evacuate on the named engine and DMA per-block
                    # to cut the serial epilogue; if this chain skipped its
                    # at-scale, fold the 1/rowsum here (rows are queries i)
                    po = pos.tile([128, 512], FP16, name=f"poL{gi}_{m}", tag="po")
                    rc = rc_tiles.pop((gi, m), None)
                    if rc is not None:
                        # scale-evac split across ACT and DVE halves
                        h = 256 if last == "split" else \
                            (512 if last == "act" else 0)
                        if h > 0:
                            nc.scalar.activation(
                                po[:, 0:h], op[:, 0:h],
                                mybir.ActivationFunctionType.Copy,
                                scale=rc[:, m:m + 1])
                        if h < 512:
                            nc.vector.tensor_scalar_mul(
                                po[:, h:512], op[:, h:512], rc[:, m:m + 1])
                    elif last == "act":
                        nc.scalar.copy(po[:], op[:])
                    else:
                        nc.vector.tensor_copy(po[:], op[:])
                    nc.sync.dma_start(
                        pout[:].rearrange("(r p) d -> p r d", p=128)
                            [:, gi * 4 + m, :], po[:])
                    return
                if m == 0:
                    pst_tiles[gi] = pos.tile([128, 4 * 512], FP16,
                                             name=f"po{gi}", tag="po")
                pst = pst_tiles[gi]
                rcs = rc_tiles.pop((gi, m))
                nc.vector.tensor_scalar_mul(
                    pst[:, m * 512:(m + 1) * 512], op[:], rcs[:, m:m + 1])
                if m == 3:
                    pout_queue.append(gi)
                    if len(pout_queue) > 2:
                        pout_flush()

            # ================= emission schedule =================
            # head: only quad-0 critical inputs before the x loads; vproj
            # and the non-critical weights drip into the gi=0 slots
            xts = {}
            nc.sync.dma_start(wqk_sb[:], wqk[:, :])
            xts[0] = xload(0)
            nc.scalar.dma_start(id8_sb[:], id8[:, :])
            nc.gpsimd.dma_start(
                b8_sb[:, 0:2048], b8[:, 0:2048])
            nc.scalar.dma_start(
                wv_sb[:].rearrange("p (kb d) -> p kb d", kb=4),
                wv[:, :].rearrange("(kb p) d -> p kb d", p=128))
            qkproj(0, xts[0])
            # rb0 q/k shuffles from three sequencers in parallel
            for t2, eng in ((0, nc.sync), (1, nc.scalar)):
                eng.dma_start(
                    q8s[:, t2 * ROWS:t2 * ROWS + 512],
                    qk8_sb[32 * t2:32 * (t2 + 1), 0:512])
            for t2, eng in ((0, nc.gpsimd), (1, nc.sync)):
                eng.dma_start(
                    k8s[:, t2 * ROWS:t2 * ROWS + 512],
                    qk8_sb[64 + 32 * t2:96 + 32 * t2, 0:512])
            # quad 0 emitted before the remaining projections so its QK
            # only waits on the rb0 chain
            qk_quad(0)
            av_memset(0)
            for rb in range(1, 4):
                xts[rb] = xload(rb)
                qkproj(rb, xts[rb])
                shuffle(0, rb=rb, parts="k")
                b8_load(rb)
            vproj(0, xts[0])
            del xts[0]
            shuffle(0, parts="q", cols=(512, 2048))
            for uq in range(4, 10):
                b8_load(uq)
            nc.scalar.dma_start(idb_sb[:], idb[:, :])
            nc.scalar.dma_start(wout_sb[:], wout[:, :])

            for u in range(1, NQUAD):
                b, t = u // 16, u % 4
                gi = u // 4
                qk_quad(u)
                if u + 9 < 16:
                    b8_load(u + 9)
                if gi == 0:
                    # batch 0's remaining v projections drip here; v block
                    # rb is first needed by the AV partial at u=rb+1
                    vproj(t, xts[t])
                    del xts[t]
                # AV partials lag their exps by one quad slot: quad t-1's
                # p is fully evacuated by the time slot t runs
                if t == 0:
                    if gi >= 1:
                        av_partial(gi - 1, 3)
                        av_post_scales(gi - 1)
                        av_memset(gi)
                else:
                    av_partial(gi, t - 1)
                    if t == 1 and gi >= 1:
                        # transposes deferred a slot so the at-scales' DVE
                        # chain never blocks the next quad's segs on PE
                        transpose_ih(gi - 1)
                if gi >= 2:
                    outproj_block(gi - 2, t)
                if gi == NGI - 1 and t >= 1:
                    # chain 14's epilogue on DVE, overlapping the final exps
                    outproj_block(gi - 1, t - 1, last="dve")
                # drip-feed next batch's projections across ALL 16 slots of
                # this batch so no slot's PE budget is overdrawn; x loads
                # run several slots ahead of their consumers
                gi_l = gi % 4
                if b + 1 < B:
                    d = gi_l * 4 + t  # 0..15
                    r0 = (b + 1) * 4
                    if 1 <= d <= 4:
                        xts[r0 + d - 1] = xload(r0 + d - 1)
                    if 4 <= d <= 11:
                        qkproj(r0 + (d - 4) // 2, xts[r0 + (d - 4) // 2],
                               half=(d - 4) % 2)
                    if d == 13:
                        shuffle(b + 1)
                    if d >= 12:
                        vproj(r0 + d - 12, xts[r0 + d - 12])
                        del xts[r0 + d - 12]

            # tail: only quad 3's AV partial + post + out-projections remain
            while pout_queue:
                pout_flush()
            gL = NGI - 1
            av_partial(gL, 3)
            outproj_block(gL - 1, 3, last="dve")
            av_post_scales(gL)
            transpose_ih(gL)
            for m in range(4):
                outproj_block(gL, m, last="act" if m % 2 == 0 else "dve")

    _legalize_waits(nc)
    return nc


def _legalize_waits(nc):
    """walrus in this container accepts at most ONE sync-wait command per
    instruction.  Tile emits coalesced multi-wait lists; split the extras
    into single-wait NoOps injected just before the instruction in its
    engine's program order (same blocking semantics, ~ns cost)."""
    n = 0
    for fn in nc.m.functions:
        for blk in fn.blocks:
            insts = list(blk.instructions)
            out = []
            for inst in insts:
                si = inst.sync_info
                if si is not None and si.on_wait and len(si.on_wait) > 1:
                    waits = list(si.on_wait)
                    for w in waits[:-1]:
                        nop = mybir.InstNoOp(
                            name=f"waitsplit_{n}",
                            engine=inst.engine,
                            ins=[],
                            outs=[],
                            bass_nofuse=True,
                            sync_info=mybir.SyncInfo(on_wait=[w], on_update=[]),
                        )
                        n += 1
                        out.append(nop)
                    inst.sync_info = mybir.SyncInfo(
                        on_wait=[waits[-1]], on_update=list(si.on_update)
                    )
                out.append(inst)
            if len(out) != len(insts):
                blk.instructions = out
    return n


def _prep_inputs(x, pos_bias, w_qkv, w_out):
    x2 = x.reshape(ROWS, D)
    xT = np.ascontiguousarray(x2.T).astype(BF16_NP)  # [512, 8192]

    wq, wk, wv = w_qkv[:, :512], w_qkv[:, 512:1024], w_qkv[:, 1024:]

    # fp8 identity (DR interleave j = p + 64*t2) for the bias-add matmul,
    # replicated on both partition halves
    id8 = np.eye(128, dtype=np.float32).reshape(2, 64, 128)  # [t2, p, m]
    id8 = np.ascontiguousarray(id8.transpose(1, 0, 2)).reshape(64, 256)
    id8 = np.concatenate([id8, id8], axis=0)  # [128, 256]
    idb = np.eye(128, dtype=np.float32)

    in_maps = []
    for c in range(NCORES):
        s = slice(c * DH, (c + 1) * DH)
        wqk_c = np.concatenate([wq[:, s], wk[:, s]], axis=1)  # [512, 128]
        # wqk_sb[p, kb*128 + m] = wqk_c[kb*128 + p, m]
        wqk_h = np.ascontiguousarray(
            wqk_c.reshape(4, 128, 128).transpose(1, 0, 2)).reshape(128, 512)
        # b8[p, u*4096 + sblk*1024 + t2*512 + il]
        #   = 8*log1p(pb[c, gi_l*512+il, (4t+sblk)*128 + p + 64*t2]); u=(gi_l,t)
        pb = 8.0 * np.log1p(pos_bias[c])          # [i, j] = [2048, 2048]
        # j index = (4t+s)*128 + p + 64*t2 with s = 2*s2hi + s2lo; partition
        # is (s2hi, p); col = gi_l*8192 + t*2048 + s2lo*1024 + t2*512 + il
        pbt = pb.T.reshape(4, 2, 2, 2, 64, 4, 512)  # [t,s2hi,s2lo,t2,p,gi_l,il]
        b8c = pbt.transpose(1, 4, 5, 0, 2, 3, 6)    # [s2hi,p,gi_l,t,s2lo,t2,il]
        b8c = np.ascontiguousarray(b8c).reshape(128, 16 * 2048).astype(F8_NP)
        in_maps.append({
            "xT": xT,
            "wqk": wqk_h.astype(BF16_NP),
            "wv": np.ascontiguousarray(wv[:, s]).astype(BF16_NP),
            "wout": np.ascontiguousarray(w_out[s, :]).astype(BF16_NP),
            "id8": id8.astype(F8_NP),
            "idb": idb.astype(BF16_NP),
            "b8": b8c,
        })
    return in_maps


def kernel(x, pos_bias, w_qkv, w_out):
    global LAST_RESULTS
    x = np.asarray(x, dtype=np.float32)
    pos_bias = np.asarray(pos_bias, dtype=np.float32)
    w_qkv = np.asarray(w_qkv, dtype=np.float32)
    w_out = np.asarray(w_out, dtype=np.float32)

    nc = build_nc()
    in_maps = _prep_inputs(x, pos_bias, w_qkv, w_out)
    res = run_bass_kernel_spmd(nc, in_maps, core_ids=list(range(NCORES)))
    LAST_RESULTS = res

    out = np.zeros((ROWS, D), dtype=np.float32)
    for c in range(NCORES):
        out += res.results[c]["pout"].astype(np.float32)
    return out.reshape(B, N, D)


if __name__ == "__main__":
    nc = build_nc()
    print("built ok")

